# revision 1
# baseline (speedup 1.0000x reference)
"""Sparse-attention 3-branch module on 8 trn2 NeuronCores.

Strategy: data-parallel over batch B=8 -> one batch element per core
(sharding_hint). Per-core forward runs the full 3-branch attention stack.
Implemented with jax pmap across the 8 neuron devices.
"""

import functools

import jax
import jax.numpy as jnp
import numpy as np

B, L, D, H, CH = 8, 2048, 128, 8, 512
EXP = 4


def _ln(x, g, b, eps=1e-6):
    mu = x.mean(-1, keepdims=True)
    v = ((x - mu) ** 2).mean(-1, keepdims=True)
    return (x - mu) * jax.lax.rsqrt(v + eps) * g + b


def _instnorm(s, eps=1e-5):
    mu = s.mean((-2, -1), keepdims=True)
    v = ((s - mu) ** 2).mean((-2, -1), keepdims=True)
    return (s - mu) * jax.lax.rsqrt(v + eps)


def _forward(c, h, w, core, p):
    # c: (1, L, CH), h/w: (1, L, D), core: (1, D, D, D)
    def cbr(x, br):
        y = jax.lax.conv_general_dilated(
            x, p['Wconv_' + br], (1, 1), ((1, 1), (1, 1)),
            dimension_numbers=('NCHW', 'OIHW', 'NCHW'))
        y = y + p['bconv_' + br][None, :, None, None]
        y = (y - p['bn_rm_' + br][None, :, None, None]) * jax.lax.rsqrt(
            p['bn_rv_' + br][None, :, None, None] + 1e-5)
        y = y * p['bn_g_' + br][None, :, None, None] + p['bn_b_' + br][None, :, None, None]
        return jax.nn.relu(y).mean(axis=1)

    core_c = _ln(cbr(jnp.transpose(core, (0, 2, 3, 1)), 'c'), p['an_g1'], p['an_b1'])
    core_h = _ln(cbr(jnp.transpose(core, (0, 3, 1, 2)), 'h'), p['an_g2'], p['an_b2'])
    core_w = _ln(cbr(core, 'w'), p['an_g3'], p['an_b3'])

    c2 = c @ p['W_l1'].T + p['b_l1']
    cq = _ln(c2, p['n_g1'], p['n_b1'])
    hq = _ln(h, p['n_g2'], p['n_b2'])
    wq = _ln(w, p['n_g3'], p['n_b3'])

    scale = jnp.sqrt(jnp.float32(CH))

    def branch(q, kv, Wq, Wk, Wv, Wo):
        Q = jnp.einsum('bld,hed->bhle', q, Wq)
        K = jnp.einsum('bkd,hed->bhke', kv, Wk)
        V = jnp.einsum('bkd,hed->bhke', kv, Wv)
        s = jnp.einsum('bhle,bhke->bhlk', Q, K) / scale
        a = jax.nn.softmax(_instnorm(s), axis=-1)
        ctx = jnp.einsum('bhlk,bhke->bhle', a, V).mean(axis=1)
        return ctx @ Wo.T

    O1 = c2 + branch(cq, core_c, p['Wq1'], p['Wk1'], p['Wv1'], p['Wo1'])
    O2 = h + branch(hq, core_h, p['Wq2'], p['Wk2'], p['Wv2'], p['Wo2'])
    O3 = w + branch(wq, core_w, p['Wq3'], p['Wk3'], p['Wv3'], p['Wo3'])

    def mlp(x, i):
        g = jax.nn.gelu(x @ p[f'ffn_W1_{i}'].T + p[f'ffn_b1_{i}'], approximate=False)
        return g @ p[f'ffn_W2_{i}'].T + p[f'ffn_b2_{i}']

    O1 = O1 + mlp(_ln(O1, p['fn_g1'], p['fn_b1']), 1)
    O2 = O2 + mlp(_ln(O2, p['fn_g2'], p['fn_b2']), 2)
    O3 = O3 + mlp(_ln(O3, p['fn_g3'], p['fn_b3']), 3)
    O1 = O1 @ p['W_l2'].T + p['b_l2']
    return (O1, O2, O3)


_pmapped = None


def _get_pmapped():
    global _pmapped
    if _pmapped is None:
        _pmapped = jax.pmap(_forward, in_axes=(0, 0, 0, 0, None),
                            devices=jax.devices()[:8])
    return _pmapped


def _run_neuron(c, h, w, core, params):
    f = _get_pmapped()
    # one batch element per core: (8, 1, L, ...) leading pmap axis
    O1, O2, O3 = f(c[:, None], h[:, None], w[:, None], core[:, None], params)
    return (np.asarray(O1).reshape(B, L, CH),
            np.asarray(O2).reshape(B, L, D),
            np.asarray(O3).reshape(B, L, D))


def _run_cpu(c, h, w, core, params):
    with jax.default_device(jax.local_devices(backend='cpu')[0]):
        f = jax.jit(_forward, backend='cpu')
        O1, O2, O3 = f(c, h, w, core, params)
        return (np.asarray(O1), np.asarray(O2), np.asarray(O3))


def kernel(c, h, w, core, params):
    c = np.asarray(c, np.float32)
    h = np.asarray(h, np.float32)
    w = np.asarray(w, np.float32)
    core = np.asarray(core, np.float32)
    params = {k: np.asarray(v, np.float32) for k, v in params.items()}
    try:
        return _run_neuron(c, h, w, core, params)
    except Exception as e:  # fall back to CPU if the device path breaks
        import sys
        print(f"kernel: neuron path failed ({e!r}); CPU fallback", file=sys.stderr)
        return _run_cpu(c, h, w, core, params)


# revision 2
# speedup vs baseline: 1.1045x; 1.1045x over previous
"""Sparse-attention 3-branch module on 8 trn2 NeuronCores.

Strategy: data-parallel over batch B=8 -> one batch element per core
(per the sharding_hint). Params are packed client-side into a single flat
f32 buffer to minimize host->device transfers over the axon tunnel, and
unpacked inside the compiled program (slicing is free at trace time).
"""

import numpy as np

B, L, D, H, CH = 8, 2048, 128, 8, 512
EXP = 4

# Fixed parameter inventory (name -> shape), matching reference.setup_inputs().
_PSPECS = []
for _br in ('c', 'h', 'w'):
    _PSPECS += [('Wconv_' + _br, (1, D, 3, 3)), ('bconv_' + _br, (1,)),
                ('bn_g_' + _br, (1,)), ('bn_b_' + _br, (1,)),
                ('bn_rm_' + _br, (1,)), ('bn_rv_' + _br, (1,))]
for _i in (1, 2, 3):
    _PSPECS += [(f'an_g{_i}', (D,)), (f'an_b{_i}', (D,)),
                (f'n_g{_i}', (D,)), (f'n_b{_i}', (D,)),
                (f'Wq{_i}', (H, D, D)), (f'Wk{_i}', (H, D, D)),
                (f'Wv{_i}', (H, D, D)), (f'Wo{_i}', (D, D)),
                (f'fn_g{_i}', (D,)), (f'fn_b{_i}', (D,)),
                (f'ffn_W1_{_i}', (EXP * D, D)), (f'ffn_b1_{_i}', (EXP * D,)),
                (f'ffn_W2_{_i}', (D, EXP * D)), (f'ffn_b2_{_i}', (D,))]
_PSPECS += [('W_l1', (D, CH)), ('b_l1', (D,)), ('W_l2', (CH, D)), ('b_l2', (CH,))]


def _pack_params(params):
    flat = [np.asarray(params[name], np.float32).reshape(-1) for name, _ in _PSPECS]
    return np.concatenate(flat)


def _unpack_params(buf):
    p, off = {}, 0
    for name, shape in _PSPECS:
        n = int(np.prod(shape))
        p[name] = buf[off:off + n].reshape(shape)
        off += n
    return p


def _forward_packed(c, h, w, core, pbuf):
    import jax
    import jax.numpy as jnp

    p = _unpack_params(pbuf)

    def _ln(x, g, b, eps=1e-6):
        mu = x.mean(-1, keepdims=True)
        v = ((x - mu) ** 2).mean(-1, keepdims=True)
        return (x - mu) * jax.lax.rsqrt(v + eps) * g + b

    def _instnorm(s, eps=1e-5):
        mu = s.mean((-2, -1), keepdims=True)
        v = ((s - mu) ** 2).mean((-2, -1), keepdims=True)
        return (s - mu) * jax.lax.rsqrt(v + eps)

    def cbr(x, br):
        y = jax.lax.conv_general_dilated(
            x, p['Wconv_' + br], (1, 1), ((1, 1), (1, 1)),
            dimension_numbers=('NCHW', 'OIHW', 'NCHW'))
        y = y + p['bconv_' + br][None, :, None, None]
        y = (y - p['bn_rm_' + br][None, :, None, None]) * jax.lax.rsqrt(
            p['bn_rv_' + br][None, :, None, None] + 1e-5)
        y = y * p['bn_g_' + br][None, :, None, None] + p['bn_b_' + br][None, :, None, None]
        return jax.nn.relu(y).mean(axis=1)

    core_c = _ln(cbr(jnp.transpose(core, (0, 2, 3, 1)), 'c'), p['an_g1'], p['an_b1'])
    core_h = _ln(cbr(jnp.transpose(core, (0, 3, 1, 2)), 'h'), p['an_g2'], p['an_b2'])
    core_w = _ln(cbr(core, 'w'), p['an_g3'], p['an_b3'])

    c2 = c @ p['W_l1'].T + p['b_l1']
    cq = _ln(c2, p['n_g1'], p['n_b1'])
    hq = _ln(h, p['n_g2'], p['n_b2'])
    wq = _ln(w, p['n_g3'], p['n_b3'])

    scale = jnp.sqrt(jnp.float32(CH))

    def branch(q, kv, Wq, Wk, Wv, Wo):
        Q = jnp.einsum('bld,hed->bhle', q, Wq)
        K = jnp.einsum('bkd,hed->bhke', kv, Wk)
        V = jnp.einsum('bkd,hed->bhke', kv, Wv)
        s = jnp.einsum('bhle,bhke->bhlk', Q, K) / scale
        a = jax.nn.softmax(_instnorm(s), axis=-1)
        ctx = jnp.einsum('bhlk,bhke->bhle', a, V).mean(axis=1)
        return ctx @ Wo.T

    O1 = c2 + branch(cq, core_c, p['Wq1'], p['Wk1'], p['Wv1'], p['Wo1'])
    O2 = h + branch(hq, core_h, p['Wq2'], p['Wk2'], p['Wv2'], p['Wo2'])
    O3 = w + branch(wq, core_w, p['Wq3'], p['Wk3'], p['Wv3'], p['Wo3'])

    def mlp(x, i):
        g = jax.nn.gelu(x @ p[f'ffn_W1_{i}'].T + p[f'ffn_b1_{i}'], approximate=False)
        return g @ p[f'ffn_W2_{i}'].T + p[f'ffn_b2_{i}']

    O1 = O1 + mlp(_ln(O1, p['fn_g1'], p['fn_b1']), 1)
    O2 = O2 + mlp(_ln(O2, p['fn_g2'], p['fn_b2']), 2)
    O3 = O3 + mlp(_ln(O3, p['fn_g3'], p['fn_b3']), 3)
    O1 = O1 @ p['W_l2'].T + p['b_l2']
    return (O1, O2, O3)


_pmapped = None


def _get_pmapped():
    global _pmapped
    if _pmapped is None:
        import jax
        _pmapped = jax.pmap(_forward_packed, in_axes=(0, 0, 0, 0, None),
                            devices=jax.devices()[:8])
    return _pmapped


def _run_neuron(c, h, w, core, pbuf):
    import numpy as _np
    f = _get_pmapped()
    O1, O2, O3 = f(c[:, None], h[:, None], w[:, None], core[:, None], pbuf)
    return (_np.asarray(O1).reshape(B, L, CH),
            _np.asarray(O2).reshape(B, L, D),
            _np.asarray(O3).reshape(B, L, D))


def _run_cpu(c, h, w, core, pbuf):
    import jax
    f = jax.jit(_forward_packed, backend='cpu')
    O1, O2, O3 = f(c, h, w, core, pbuf)
    return (np.asarray(O1), np.asarray(O2), np.asarray(O3))


def kernel(c, h, w, core, params):
    c = np.ascontiguousarray(np.asarray(c, np.float32))
    h = np.ascontiguousarray(np.asarray(h, np.float32))
    w = np.ascontiguousarray(np.asarray(w, np.float32))
    core = np.ascontiguousarray(np.asarray(core, np.float32))
    pbuf = _pack_params(params)
    try:
        return _run_neuron(c, h, w, core, pbuf)
    except Exception as e:
        import sys
        print(f"kernel: neuron path failed ({e!r}); CPU fallback", file=sys.stderr)
        return _run_cpu(c, h, w, core, pbuf)
